# revision 39
# baseline (speedup 1.0000x reference)
"""Multi-head attention (COAMultiHeadAttention) on 8 Trainium2 NeuronCores.

Sharding: batch x head-group. Core c (0..7) handles batch b = c//4 and head
group g = c%4 (4 of 16 heads, i.e. a 256-wide slice of the 1024-dim model).
Each core:
  - projects its batch's q/k/v through its 256-row weight slices (Q^T/K^T
    d-major, V token-major),
  - runs attention for its 4 heads in a score-transposed (k-part, q-free)
    layout: QK^T -> exp on ScalarE (PSUM->SBUF, bf16) -> P~V with an extra
    ones column in V that yields the softmax denominators for free,
  - normalizes via GPSIMD partition-broadcast + VectorE reciprocal/multiply,
  - computes its partial output projection att_n @ wo[:, slice]^T.
Host sums the 4 partials per batch and adds bo.

Everything the device sees is pre-laid-out by the host (transposes, head
slicing, bias replication), so the device program is identical on all cores.
"""

import os

import ml_dtypes
import numpy as np

import concourse.bass as bass
import concourse.mybir as mybir
import concourse.tile as tile
from concourse import bacc, bass_utils

F32 = mybir.dt.float32
F32R = mybir.dt.float32r
BF16 = mybir.dt.bfloat16
AT = mybir.ActivationFunctionType
ALU = mybir.AluOpType

B = 2
T = 2048
D = 1024
N_HEADS = 16
HEAD_DIM = 64
N_CORES = 8
S = 256            # per-core slice of the model dim (4 heads)
NHL = 4            # heads per core
P = 128
DC = D // P        # 8 contraction chunks for the projections
TC = T // P        # 16 token chunks
SCALE = 1.0 / np.sqrt(HEAD_DIM)

_CACHE = {}
LAST_STATS = {}


def _patch_walrus_flags():
    """Enable walrus's LDWEIGHTS optimization (dedupe/pipeline weight loads).
    bass_utils hardcodes --enable-ldw-opt=false; without the opt every MATMUL
    serially waits ~140ns for its own LDWEIGHTS, which costs ~100us here."""
    if getattr(bass_utils, "_ldw_patched", False):
        return
    orig = bass_utils.run_command

    def patched(argv, **kw):
        argv = ["--enable-ldw-opt=true" if a == "--enable-ldw-opt=false" else a
                for a in argv]
        return orig(argv, **kw)

    bass_utils.run_command = patched
    bass_utils._ldw_patched = True


def _build_program():
    nc = bacc.Bacc("TRN2", target_bir_lowering=False, debug=False)

    xq_d = nc.dram_tensor("xq", [P, DC, T], BF16, kind="ExternalInput").ap()
    xk_d = nc.dram_tensor("xk", [P, DC, T], BF16, kind="ExternalInput").ap()
    xv_d = nc.dram_tensor("xv", [P, DC, T], BF16, kind="ExternalInput").ap()
    wqt_d = nc.dram_tensor("wqt", [P, DC, S], BF16, kind="ExternalInput").ap()
    wkt_d = nc.dram_tensor("wkt", [P, DC, S], BF16, kind="ExternalInput").ap()
    wvt_d = nc.dram_tensor("wvt", [P, DC, S], BF16, kind="ExternalInput").ap()
    bq_d = nc.dram_tensor("bq", [P, 2], F32, kind="ExternalInput").ap()
    bk_d = nc.dram_tensor("bk", [P, 2], F32, kind="ExternalInput").ap()
    bv_d = nc.dram_tensor("bv", [P, NHL, HEAD_DIM], F32, kind="ExternalInput").ap()
    wot_d = nc.dram_tensor("wot", [P, 2, D], BF16, kind="ExternalInput").ap()
    out_d = nc.dram_tensor("out_part", [TC, P, D], BF16, kind="ExternalOutput").ap()
    sums_d = nc.dram_tensor("sums_scr", [NHL, T], F32).ap()
    rsums_d = nc.dram_tensor("rsums_scr", [NHL, T], F32).ap()

    with tile.TileContext(nc) as tc:
        _body(tc, xq_d, xk_d, xv_d, wqt_d, wkt_d, wvt_d,
              bq_d, bk_d, bv_d, wot_d, out_d, sums_d, rsums_d)
    nc.compile()
    return nc


def _body(tc, xq_d, xk_d, xv_d, wqt_d, wkt_d, wvt_d, bq_d, bk_d, bv_d, wot_d,
          out_d, sums_d, rsums_d):
    nc = tc.nc

    from contextlib import ExitStack
    with ExitStack() as ctx:
        pers = ctx.enter_context(tc.tile_pool(name="pers", bufs=1))
        qt_sb = pers.tile([P, 2, T], BF16, tag="qt")
        kt_sb = pers.tile([P, 2, T], BF16, tag="kt")
        v_sb = pers.tile([P, TC, NHL, 68], BF16, tag="v")
        attn_sb = pers.tile([P, 2, T], BF16, tag="attn")
        wot_sb = pers.tile([P, 2, D], BF16, tag="wot")
        bq_sb = pers.tile([P, 2], F32, tag="bq")
        bk_sb = pers.tile([P, 2], F32, tag="bk")
        bv_sb = pers.tile([P, NHL, HEAD_DIM], F32, tag="bv")
        zero_sb = pers.tile([P, 1], F32, tag="zero")
        scr_sb = pers.tile([P, 1], F32, tag="scr")

        nc.sync.dma_start(wot_sb[:], wot_d[:])
        nc.sync.dma_start(bq_sb[:], bq_d[:])
        nc.sync.dma_start(bk_sb[:], bk_d[:])
        nc.sync.dma_start(bv_sb[:], bv_d[:])
        nc.vector.memset(zero_sb[:], 0.0)
        # Preload the exp table set (~2.7us) during phase A so the first real
        # exp doesn't stall the attention pipeline right when HAM's idle
        # window could re-throttle the PE clock.
        nc.scalar.activation(scr_sb[:], zero_sb[:], AT.Exp,
                             bias=zero_sb[:, 0:1], scale=1.0)
        # ones column for the P~V sum trick (col 64 of every (tchunk, head) slot)
        nc.vector.memset(v_sb[:, :, :, 64:65], 1.0)

        # ---------------- Phase A: projections ----------------
        with tc.tile_pool(name="xw", bufs=1) as xw, \
             tc.tile_pool(name="pjps", bufs=4, space="PSUM") as pjps:
            wq_sb = xw.tile([P, DC, S], BF16, tag="wq")
            wk_sb = xw.tile([P, DC, S], BF16, tag="wk")
            wv_sb = xw.tile([P, DC, S], BF16, tag="wv")
            xq_sb = xw.tile([P, DC, T], BF16, tag="xq")
            xk_sb = xw.tile([P, DC, T], BF16, tag="xk")
            xv_sb = xw.tile([P, DC, T], BF16, tag="xv")

            for c in range(DC):
                nc.sync.dma_start(wk_sb[:, c], wkt_d[:, c])
                nc.sync.dma_start(xk_sb[:, c], xk_d[:, c])
            for c in range(DC):
                nc.sync.dma_start(wq_sb[:, c], wqt_d[:, c])
                nc.sync.dma_start(xq_sb[:, c], xq_d[:, c])
            for c in range(DC):
                nc.sync.dma_start(wv_sb[:, c], wvt_d[:, c])
                nc.sync.dma_start(xv_sb[:, c], xv_d[:, c])

            # K^T and Q^T projections: (256, T) d-major, bf16 + bias.
            # d8-outer so compute starts as soon as the first 128-row chunk
            # of x lands, with all four n-groups accumulating in PSUM.
            def proj_kq(x_sb, w_sb, b_sb, dst, m):
                for n in range(4):
                    ps = pjps.tile([P, 512], F32, tag="pj")
                    for d8 in range(DC):
                        nc.tensor.matmul(
                            ps[:],
                            lhsT=w_sb[:, d8, m * P:(m + 1) * P],
                            rhs=x_sb[:, d8, n * 512:(n + 1) * 512],
                            start=(d8 == 0), stop=(d8 == DC - 1))
                    nc.vector.tensor_scalar(
                        dst[:, m, n * 512:(n + 1) * 512], ps[:],
                        b_sb[:, m:m + 1], None, op0=ALU.add)

            proj_kq(xk_sb, wk_sb, bk_sb, kt_sb, 0)
            proj_kq(xk_sb, wk_sb, bk_sb, kt_sb, 1)
            proj_kq(xq_sb, wq_sb, bq_sb, qt_sb, 0)
            proj_kq(xq_sb, wq_sb, bq_sb, qt_sb, 1)

            # V projection: token-major (T, 256) bf16 + bias
            for t16 in range(TC):
                ps = pjps.tile([P, S], F32, tag="pj")
                for d8 in range(DC):
                    nc.tensor.matmul(
                        ps[:],
                        lhsT=xv_sb[:, d8, t16 * P:(t16 + 1) * P],
                        rhs=wv_sb[:, d8, :],
                        start=(d8 == 0), stop=(d8 == DC - 1))
                nc.vector.tensor_tensor(
                    v_sb[:, t16, :, 0:64],
                    ps[:].rearrange("p (h x) -> p h x", h=NHL),
                    bv_sb[:], op=ALU.add)

        # ---------------- Phase B: attention ----------------
        # Heads are processed in pairs (0,1) and (2,3). Within a pair, head A
        # lives at partitions 0-63 and head B at 64-127, so interleaved QK
        # matmuls alternate PE row groups — the PE's reorder window can then
        # pull each LDWEIGHTS ahead of the other head's in-flight matmul.
        # q is processed in halves of 1024 so the four PSUM accumulators
        # (2 heads x (ST tile + att tile)) fit in the 8 banks exactly.
        # The PV matmuls lag one step behind QK/exp, and each block's final
        # PV + evacuations are emitted after the NEXT block's first QK/exp —
        # so the ScalarE exp stream never stalls at block boundaries.
        QH = 1024
        with tc.tile_pool(name="stp", bufs=2, space="PSUM") as stp, \
             tc.tile_pool(name="attp", bufs=2, space="PSUM") as attp, \
             tc.tile_pool(name="ptp", bufs=4) as ptp, \
             tc.tile_pool(name="asb", bufs=4) as asbp, \
             tc.tile_pool(name="brd", bufs=2) as brdp, \
             tc.tile_pool(name="rcp", bufs=4) as rcpp:
            pending_pv = []

            def emit_pv(ent):
                att_A, att_B, mh, i, pt_A, pt_B = ent
                for n in range(2):
                    ns = slice(n * 512, (n + 1) * 512)
                    nc.tensor.matmul(
                        att_A[:, ns], lhsT=v_sb[:, i, 2 * mh, 0:65],
                        rhs=pt_A[:, ns], start=(i == 0), stop=(i == TC - 1))
                    nc.tensor.matmul(
                        att_B[:, ns], lhsT=v_sb[:, i, 2 * mh + 1, 0:65],
                        rhs=pt_B[:, ns], start=(i == 0), stop=(i == TC - 1))

            def emit_block_tail(ent):
                att_A, att_B, mh, jh = ent
                q0 = jh * QH
                # Evacuate both PSUM accumulators first (frees the banks for
                # the next block before the slower norm chains run — DVE is
                # strict FIFO, so norm DMA-waits must not sit between them).
                attsbs = []
                for hb, att_ps in ((0, att_A), (1, att_B)):
                    attsb = asbp.tile([65, QH], F32, tag="attsb",
                                      name=f"attsb{hb}")
                    nc.vector.tensor_copy(attsb[:], att_ps[:])
                    attsbs.append(attsb)
                for hb, attsb in ((0, attsbs[0]), (1, attsbs[1])):
                    h = 2 * mh + hb
                    ph = hb * 64
                    # Softmax denominators: reciprocal in partition-major
                    # shape (DVE cost scales with free size only), then a
                    # partition broadcast — both via DRAM bounces, since
                    # SBUF APs cannot have a zero partition step.
                    nc.sync.dma_start(sums_d[h:h + 1, q0:q0 + QH],
                                      attsb[64:65, :])
                    sp = rcpp.tile([P, QH // P], F32, tag="sp")
                    nc.sync.dma_start(
                        sp[:], sums_d[h, q0:q0 + QH].rearrange(
                            "(p f) -> p f", p=P))
                    rp = rcpp.tile([P, QH // P], F32, tag="rp")
                    nc.vector.reciprocal(rp[:], sp[:])
                    nc.sync.dma_start(
                        rsums_d[h, q0:q0 + QH].rearrange("(p f) -> p f", p=P),
                        rp[:])
                    rc = brdp.tile([64, QH], F32, tag="rc")
                    nc.sync.dma_start(
                        rc[:], rsums_d[h:h + 1, q0:q0 + QH].broadcast_to((64, QH)))
                    nc.vector.tensor_tensor(
                        attn_sb[ph:ph + 64, mh, q0:q0 + QH],
                        attsb[0:64, :], rc[:], op=ALU.mult)

            pending_tail = None
            for bi, (mh, jh) in enumerate([(m, j) for m in range(2)
                                           for j in range(2)]):
                q0 = jh * QH
                att_A = attp.tile([65, QH], F32, tag="att", name="att_A")
                att_B = attp.tile([65, QH], F32, tag="att", name="att_B")
                for i in range(TC):
                    st_A = stp.tile([P, QH], F32, tag="st", name="st_A")
                    st_B = stp.tile([P, QH], F32, tag="st", name="st_B")
                    for n in range(2):
                        ns = slice(n * 512, (n + 1) * 512)
                        qs = slice(q0 + n * 512, q0 + (n + 1) * 512)
                        nc.tensor.matmul(
                            st_A[:, ns],
                            lhsT=kt_sb[0:64, mh, i * P:(i + 1) * P],
                            rhs=qt_sb[0:64, mh, qs],
                            start=True, stop=True)
                        nc.tensor.matmul(
                            st_B[:, ns],
                            lhsT=kt_sb[64:128, mh, i * P:(i + 1) * P],
                            rhs=qt_sb[64:128, mh, qs],
                            start=True, stop=True)
                    if bi == 0 and i == 0:
                        # Full-row keep-warm matmuls: bridge the exp
                        # pipeline-fill stall so the PE's HAM activity window
                        # never sees >3.4us idle (which would gate the clock
                        # to 1.2 GHz for the whole phase). The i=0 PV has
                        # start=True, which overwrites this garbage.
                        for wmm in range(8):
                            nc.tensor.matmul(
                                att_A[:, (wmm % 2) * 512:(wmm % 2) * 512 + 512],
                                lhsT=v_sb[:, 0, 2 * mh, 0:65],
                                rhs=kt_sb[:, mh, 0:512],
                                start=True, stop=True)
                    pt_A = ptp.tile([P, QH], BF16, tag="pt", name="pt_A")
                    nc.scalar.activation(pt_A[:], st_A[:], AT.Exp,
                                         bias=zero_sb[:, 0:1], scale=float(SCALE))
                    pt_B = ptp.tile([P, QH], BF16, tag="pt", name="pt_B")
                    nc.scalar.activation(pt_B[:], st_B[:], AT.Exp,
                                         bias=zero_sb[:, 0:1], scale=float(SCALE))
                    pending_pv.append((att_A, att_B, mh, i, pt_A, pt_B))
                    if len(pending_pv) > 1:
                        emit_pv(pending_pv.pop(0))
                    if i == 1 and pending_tail is not None:
                        emit_block_tail(pending_tail)
                        pending_tail = None
                pending_tail = (att_A, att_B, mh, jh)
            while pending_pv:
                emit_pv(pending_pv.pop(0))
            emit_block_tail(pending_tail)

        # ---------------- Phase C: output projection (partial) ----------------
        with tc.tile_pool(name="ops", bufs=3, space="PSUM") as ops, \
             tc.tile_pool(name="owm", bufs=1, space="PSUM") as owm, \
             tc.tile_pool(name="osb", bufs=4) as osb:
            wmt = owm.tile([P, 512], F32, tag="wmt")
            for m in range(TC):
                po = ops.tile([P, D], F32, tag="po")
                # keep-warm matmul: the evac-paced pipeline here has PE gaps
                # that otherwise let HAM re-throttle the clock
                nc.tensor.matmul(wmt[:], lhsT=attn_sb[:, 0, 0:P],
                                 rhs=wot_sb[:, 0, 0:512], start=True, stop=True)
                for sc in range(2):
                    for n in range(2):
                        nc.tensor.matmul(
                            po[:, n * 512:(n + 1) * 512],
                            lhsT=attn_sb[:, sc, m * P:(m + 1) * P],
                            rhs=wot_sb[:, sc, n * 512:(n + 1) * 512],
                            start=(sc == 0), stop=(sc == 1))
                ob = osb.tile([P, D], BF16, tag="ob")
                if m % 2 == 0:
                    nc.scalar.copy(ob[:], po[:])
                else:
                    nc.vector.tensor_copy(ob[:], po[:])
                nc.sync.dma_start(out_d[m], ob[:])


def _shard_inputs(query, key, value, wq, bq, wk, bk, wv, bv, wo):
    """Build the 8 per-core input maps (all host-side numpy)."""
    bf16 = ml_dtypes.bfloat16
    in_maps = []

    def fold_dmajor(a_t, inner):
        # (D, inner) -> [P, DC, inner]
        return np.ascontiguousarray(
            a_t.reshape(DC, P, inner).transpose(1, 0, 2))

    xs = {}
    for b in range(B):
        for name, x in (("xq", query), ("xk", key), ("xv", value)):
            xt = np.ascontiguousarray(x[b].T).astype(bf16)  # (D, T)
            xs[(name, b)] = fold_dmajor(xt, T)

    for c in range(N_CORES):
        b, g = divmod(c, NHL)
        gs = g * S
        wq_g = wq[gs:gs + S]          # (S, D)
        wk_g = wk[gs:gs + S]
        wv_g = wv[gs:gs + S]
        wo_g = wo[:, gs:gs + S]       # (D, S)
        m = {
            "xq": xs[("xq", b)],
            "xk": xs[("xk", b)],
            "xv": xs[("xv", b)],
            "wqt": fold_dmajor(np.ascontiguousarray(wq_g.T).astype(bf16), S),
            "wkt": fold_dmajor(np.ascontiguousarray(wk_g.T).astype(bf16), S),
            "wvt": fold_dmajor(np.ascontiguousarray(wv_g.T).astype(bf16), S),
            "bq": np.ascontiguousarray(
                bq[gs:gs + S].reshape(2, P).T).astype(np.float32),
            "bk": np.ascontiguousarray(
                bk[gs:gs + S].reshape(2, P).T).astype(np.float32),
            "bv": np.ascontiguousarray(np.broadcast_to(
                bv[gs:gs + S].reshape(NHL, HEAD_DIM), (P, NHL, HEAD_DIM))
            ).astype(np.float32),
            "wot": np.ascontiguousarray(
                wo_g.T.reshape(2, P, D).transpose(1, 0, 2)).astype(bf16),
        }
        in_maps.append(m)
    return in_maps


def _reference_numpy(query, key, value, mask, wq, bq, wk, bk, wv, bv, wo, bo):
    """Pure-numpy fallback for non-trivial masks (never hit for spec inputs)."""
    def lin(x, w, b):
        return np.einsum("btd,od->bto", x, w) + b
    Bq, Tq, _ = query.shape
    Q = lin(query, wq, bq).reshape(Bq, Tq, N_HEADS, HEAD_DIM).transpose(0, 2, 1, 3)
    K = lin(key, wk, bk).reshape(Bq, Tq, N_HEADS, HEAD_DIM).transpose(0, 2, 1, 3)
    V = lin(value, wv, bv).reshape(Bq, Tq, N_HEADS, HEAD_DIM).transpose(0, 2, 1, 3)
    scores = np.einsum("bhqd,bhkd->bhqk", Q, K) * SCALE
    scores = np.where(mask[:, None, :, :] == 0, -np.inf, scores)
    scores = scores - scores.max(axis=-1, keepdims=True)
    e = np.exp(scores)
    probs = e / e.sum(axis=-1, keepdims=True)
    att = np.einsum("bhqk,bhkd->bhqd", probs, V)
    att = att.transpose(0, 2, 1, 3).reshape(Bq, Tq, N_HEADS * HEAD_DIM)
    return (np.einsum("btd,od->bto", att, wo) + bo).astype(np.float32)


def _enable_local_tracing():
    """Make bass_utils' axon NTFF-trace path work in this container:
    register the ctypes profile hook under the missing antenv.axon_hooks
    name and keep artifacts local instead of uploading."""
    import sys
    import types
    try:
        import antenv.axon_hooks  # noqa: F401
    except Exception:
        try:
            from trn_agent_boot.trn_boot import _ntff_profile_via_ctypes
            hook = _ntff_profile_via_ctypes("/opt/axon/libaxon_pjrt.so")
            if hook is None:
                return False
            holder = {"hook": hook}
            m2 = types.ModuleType("antenv.axon_hooks")
            m2.get_axon_ntff_profile_hook = lambda: holder["hook"]
            m2.set_axon_ntff_profile_hook = lambda h: holder.update(hook=h)
            if "antenv" not in sys.modules:
                m1 = types.ModuleType("antenv")
                m1.axon_hooks = m2
                sys.modules["antenv"] = m1
            else:
                sys.modules["antenv"].axon_hooks = m2
            sys.modules["antenv.axon_hooks"] = m2
        except Exception:
            return False
    bass_utils.upload_artifacts = lambda tmpdir: tmpdir
    return True


def kernel(query, key, value, mask, wq, bq, wk, bk, wv, bv, wo, bo):
    query = np.asarray(query, np.float32)
    key = np.asarray(key, np.float32)
    value = np.asarray(value, np.float32)
    wq_, bq_ = np.asarray(wq, np.float32), np.asarray(bq, np.float32)
    wk_, bk_ = np.asarray(wk, np.float32), np.asarray(bk, np.float32)
    wv_, bv_ = np.asarray(wv, np.float32), np.asarray(bv, np.float32)
    wo_, bo_ = np.asarray(wo, np.float32), np.asarray(bo, np.float32)
    mask_np = np.asarray(mask)

    if not np.all(mask_np != 0):
        # Spec inputs always have an all-ones mask; keep a correct fallback.
        return _reference_numpy(query, key, value, mask_np, wq_, bq_,
                                wk_, bk_, wv_, bv_, wo_, bo_)

    # Experimental only: walrus's LDW opt rejects some of our weight loads.
    if os.environ.get("KERNEL_LDW_OPT", "0") == "1":
        _patch_walrus_flags()

    if "prog" not in _CACHE:
        _CACHE["prog"] = _build_program()
    nc = _CACHE["prog"]

    in_maps = _shard_inputs(query, key, value, wq_, bq_, wk_, bk_, wv_, bv_, wo_)

    trace = os.environ.get("KERNEL_TRACE", "0") == "1"
    kw = {}
    if trace:
        trace = _enable_local_tracing()
        if trace:
            tdir = os.environ.get("KERNEL_TRACE_DIR")
            if tdir:
                os.makedirs(tdir, exist_ok=True)
                kw["tmpdir"] = tdir
    try:
        res = bass_utils.run_bass_kernel_spmd(
            nc, in_maps, core_ids=list(range(N_CORES)), trace=trace, **kw)
    except Exception:
        if not trace:
            raise
        import traceback
        traceback.print_exc()
        res = bass_utils.run_bass_kernel_spmd(
            nc, in_maps, core_ids=list(range(N_CORES)), trace=False)

    LAST_STATS.clear()
    LAST_STATS["exec_time_ns"] = res.exec_time_ns
    LAST_STATS["profile_json"] = res.profile_json
    if res.instructions_and_trace is not None:
        LAST_STATS["trace_url"] = res.instructions_and_trace[1]

    out = np.empty((B, T, D), np.float32)
    for b in range(B):
        acc = None
        for g in range(NHL):
            part = res.results[b * NHL + g]["out_part"].reshape(T, D)
            acc = part.copy() if acc is None else acc + part
        out[b] = acc + bo_
    return out


# revision 40
# speedup vs baseline: 1.0020x; 1.0020x over previous
"""Multi-head attention (COAMultiHeadAttention) on 8 Trainium2 NeuronCores.

Sharding: batch x head-group. Core c (0..7) handles batch b = c//4 and head
group g = c%4 (4 of 16 heads, i.e. a 256-wide slice of the 1024-dim model).
Each core:
  - projects its batch's q/k/v through its 256-row weight slices (Q^T/K^T
    d-major, V token-major),
  - runs attention for its 4 heads in a score-transposed (k-part, q-free)
    layout: QK^T -> exp on ScalarE (PSUM->SBUF, bf16) -> P~V with an extra
    ones column in V that yields the softmax denominators for free,
  - normalizes via GPSIMD partition-broadcast + VectorE reciprocal/multiply,
  - computes its partial output projection att_n @ wo[:, slice]^T.
Host sums the 4 partials per batch and adds bo.

Everything the device sees is pre-laid-out by the host (transposes, head
slicing, bias replication), so the device program is identical on all cores.
"""

import os

import ml_dtypes
import numpy as np

import concourse.bass as bass
import concourse.mybir as mybir
import concourse.tile as tile
from concourse import bacc, bass_utils

F32 = mybir.dt.float32
F32R = mybir.dt.float32r
BF16 = mybir.dt.bfloat16
AT = mybir.ActivationFunctionType
ALU = mybir.AluOpType

B = 2
T = 2048
D = 1024
N_HEADS = 16
HEAD_DIM = 64
N_CORES = 8
S = 256            # per-core slice of the model dim (4 heads)
NHL = 4            # heads per core
P = 128
DC = D // P        # 8 contraction chunks for the projections
TC = T // P        # 16 token chunks
SCALE = 1.0 / np.sqrt(HEAD_DIM)

_CACHE = {}
LAST_STATS = {}


def _patch_walrus_flags():
    """Enable walrus's LDWEIGHTS optimization (dedupe/pipeline weight loads).
    bass_utils hardcodes --enable-ldw-opt=false; without the opt every MATMUL
    serially waits ~140ns for its own LDWEIGHTS, which costs ~100us here."""
    if getattr(bass_utils, "_ldw_patched", False):
        return
    orig = bass_utils.run_command

    def patched(argv, **kw):
        argv = ["--enable-ldw-opt=true" if a == "--enable-ldw-opt=false" else a
                for a in argv]
        return orig(argv, **kw)

    bass_utils.run_command = patched
    bass_utils._ldw_patched = True


def _build_program():
    nc = bacc.Bacc("TRN2", target_bir_lowering=False, debug=False)

    xq_d = nc.dram_tensor("xq", [P, DC, T], BF16, kind="ExternalInput").ap()
    xk_d = nc.dram_tensor("xk", [P, DC, T], BF16, kind="ExternalInput").ap()
    xv_d = nc.dram_tensor("xv", [P, DC, T], BF16, kind="ExternalInput").ap()
    wqt_d = nc.dram_tensor("wqt", [P, DC, S], BF16, kind="ExternalInput").ap()
    wkt_d = nc.dram_tensor("wkt", [P, DC, S], BF16, kind="ExternalInput").ap()
    wvt_d = nc.dram_tensor("wvt", [P, DC, S], BF16, kind="ExternalInput").ap()
    bq_d = nc.dram_tensor("bq", [P, 2], F32, kind="ExternalInput").ap()
    bk_d = nc.dram_tensor("bk", [P, 2], F32, kind="ExternalInput").ap()
    bv_d = nc.dram_tensor("bv", [P, NHL, HEAD_DIM], F32, kind="ExternalInput").ap()
    wot_d = nc.dram_tensor("wot", [P, 2, D], BF16, kind="ExternalInput").ap()
    out_d = nc.dram_tensor("out_part", [TC, P, D], BF16, kind="ExternalOutput").ap()
    sums_d = nc.dram_tensor("sums_scr", [NHL, T], F32).ap()
    rsums_d = nc.dram_tensor("rsums_scr", [NHL, T], F32).ap()

    with tile.TileContext(nc) as tc:
        _body(tc, xq_d, xk_d, xv_d, wqt_d, wkt_d, wvt_d,
              bq_d, bk_d, bv_d, wot_d, out_d, sums_d, rsums_d)
    nc.compile()
    return nc


def _body(tc, xq_d, xk_d, xv_d, wqt_d, wkt_d, wvt_d, bq_d, bk_d, bv_d, wot_d,
          out_d, sums_d, rsums_d):
    nc = tc.nc

    from contextlib import ExitStack
    with ExitStack() as ctx:
        pers = ctx.enter_context(tc.tile_pool(name="pers", bufs=1))
        qt_sb = pers.tile([P, 2, T], BF16, tag="qt")
        kt_sb = pers.tile([P, 2, T], BF16, tag="kt")
        v_sb = pers.tile([P, TC, NHL, 68], BF16, tag="v")
        attn_sb = pers.tile([P, 2, T], BF16, tag="attn")
        wot_sb = pers.tile([P, 2, D], BF16, tag="wot")
        bq_sb = pers.tile([P, 2], F32, tag="bq")
        bk_sb = pers.tile([P, 2], F32, tag="bk")
        bv_sb = pers.tile([P, NHL, HEAD_DIM], F32, tag="bv")
        zero_sb = pers.tile([P, 1], F32, tag="zero")
        scr_sb = pers.tile([P, 1], F32, tag="scr")

        nc.sync.dma_start(wot_sb[:], wot_d[:])
        nc.sync.dma_start(bq_sb[:], bq_d[:])
        nc.sync.dma_start(bk_sb[:], bk_d[:])
        nc.sync.dma_start(bv_sb[:], bv_d[:])
        nc.vector.memset(zero_sb[:], 0.0)
        # Preload the exp table set (~2.7us) during phase A so the first real
        # exp doesn't stall the attention pipeline right when HAM's idle
        # window could re-throttle the PE clock.
        nc.scalar.activation(scr_sb[:], zero_sb[:], AT.Exp,
                             bias=zero_sb[:, 0:1], scale=1.0)
        # ones column for the P~V sum trick (col 64 of every (tchunk, head) slot)
        nc.vector.memset(v_sb[:, :, :, 64:65], 1.0)

        # ---------------- Phase A: projections ----------------
        with tc.tile_pool(name="xw", bufs=1) as xw, \
             tc.tile_pool(name="pjps", bufs=4, space="PSUM") as pjps:
            wq_sb = xw.tile([P, DC, S], BF16, tag="wq")
            wk_sb = xw.tile([P, DC, S], BF16, tag="wk")
            wv_sb = xw.tile([P, DC, S], BF16, tag="wv")
            xq_sb = xw.tile([P, DC, T], BF16, tag="xq")
            xk_sb = xw.tile([P, DC, T], BF16, tag="xk")
            xv_sb = xw.tile([P, DC, T], BF16, tag="xv")

            for c in range(DC):
                nc.sync.dma_start(wk_sb[:, c], wkt_d[:, c])
                nc.sync.dma_start(xk_sb[:, c], xk_d[:, c])
            for c in range(DC):
                nc.sync.dma_start(wq_sb[:, c], wqt_d[:, c])
                nc.sync.dma_start(xq_sb[:, c], xq_d[:, c])
            for c in range(DC):
                nc.sync.dma_start(wv_sb[:, c], wvt_d[:, c])
                nc.sync.dma_start(xv_sb[:, c], xv_d[:, c])

            # K^T and Q^T projections: (256, T) d-major, bf16 + bias.
            # d8-outer so compute starts as soon as the first 128-row chunk
            # of x lands, with all four n-groups accumulating in PSUM.
            def proj_kq(x_sb, w_sb, b_sb, dst, m):
                for n in range(4):
                    ps = pjps.tile([P, 512], F32, tag="pj")
                    for d8 in range(DC):
                        nc.tensor.matmul(
                            ps[:],
                            lhsT=w_sb[:, d8, m * P:(m + 1) * P],
                            rhs=x_sb[:, d8, n * 512:(n + 1) * 512],
                            start=(d8 == 0), stop=(d8 == DC - 1))
                    nc.vector.tensor_scalar(
                        dst[:, m, n * 512:(n + 1) * 512], ps[:],
                        b_sb[:, m:m + 1], None, op0=ALU.add)

            proj_kq(xk_sb, wk_sb, bk_sb, kt_sb, 0)
            proj_kq(xk_sb, wk_sb, bk_sb, kt_sb, 1)
            proj_kq(xq_sb, wq_sb, bq_sb, qt_sb, 0)
            proj_kq(xq_sb, wq_sb, bq_sb, qt_sb, 1)

            # V projection: token-major (T, 256) bf16 + bias
            for t16 in range(TC):
                ps = pjps.tile([P, S], F32, tag="pj")
                for d8 in range(DC):
                    nc.tensor.matmul(
                        ps[:],
                        lhsT=xv_sb[:, d8, t16 * P:(t16 + 1) * P],
                        rhs=wv_sb[:, d8, :],
                        start=(d8 == 0), stop=(d8 == DC - 1))
                nc.vector.tensor_tensor(
                    v_sb[:, t16, :, 0:64],
                    ps[:].rearrange("p (h x) -> p h x", h=NHL),
                    bv_sb[:], op=ALU.add)

        # ---------------- Phase B: attention ----------------
        # Heads are processed in pairs (0,1) and (2,3). Within a pair, head A
        # lives at partitions 0-63 and head B at 64-127, so interleaved QK
        # matmuls alternate PE row groups — the PE's reorder window can then
        # pull each LDWEIGHTS ahead of the other head's in-flight matmul.
        # q is processed in halves of 1024 so the four PSUM accumulators
        # (2 heads x (ST tile + att tile)) fit in the 8 banks exactly.
        # The PV matmuls lag one step behind QK/exp, and each block's final
        # PV + evacuations are emitted after the NEXT block's first QK/exp —
        # so the ScalarE exp stream never stalls at block boundaries.
        QH = 1024
        with tc.tile_pool(name="stp", bufs=2, space="PSUM") as stp, \
             tc.tile_pool(name="attp", bufs=2, space="PSUM") as attp, \
             tc.tile_pool(name="ptp", bufs=4) as ptp, \
             tc.tile_pool(name="asb", bufs=4) as asbp, \
             tc.tile_pool(name="brd", bufs=2) as brdp, \
             tc.tile_pool(name="rcp", bufs=4) as rcpp:
            pending_pv = []

            def emit_pv(ent):
                att_A, att_B, mh, i, pt_A, pt_B = ent
                for n in range(2):
                    ns = slice(n * 512, (n + 1) * 512)
                    nc.tensor.matmul(
                        att_A[:, ns], lhsT=v_sb[:, i, 2 * mh, 0:65],
                        rhs=pt_A[:, ns], start=(i == 0), stop=(i == TC - 1))
                    nc.tensor.matmul(
                        att_B[:, ns], lhsT=v_sb[:, i, 2 * mh + 1, 0:65],
                        rhs=pt_B[:, ns], start=(i == 0), stop=(i == TC - 1))

            def emit_block_tail(ent):
                att_A, att_B, mh, jh = ent
                q0 = jh * QH
                # Evacuate both PSUM accumulators first (frees the banks for
                # the next block before the slower norm chains run — DVE is
                # strict FIFO, so norm DMA-waits must not sit between them).
                attsbs = []
                for hb, att_ps in ((0, att_A), (1, att_B)):
                    attsb = asbp.tile([65, QH], F32, tag="attsb",
                                      name=f"attsb{hb}")
                    nc.vector.tensor_copy(attsb[:], att_ps[:])
                    attsbs.append(attsb)
                for hb, attsb in ((0, attsbs[0]), (1, attsbs[1])):
                    h = 2 * mh + hb
                    ph = hb * 64
                    # Softmax denominators: reciprocal in partition-major
                    # shape (DVE cost scales with free size only), then a
                    # partition broadcast — both via DRAM bounces, since
                    # SBUF APs cannot have a zero partition step.
                    nc.sync.dma_start(sums_d[h:h + 1, q0:q0 + QH],
                                      attsb[64:65, :])
                    sp = rcpp.tile([P, QH // P], F32, tag="sp")
                    nc.sync.dma_start(
                        sp[:], sums_d[h, q0:q0 + QH].rearrange(
                            "(p f) -> p f", p=P))
                    rp = rcpp.tile([P, QH // P], F32, tag="rp")
                    nc.vector.reciprocal(rp[:], sp[:])
                    nc.sync.dma_start(
                        rsums_d[h, q0:q0 + QH].rearrange("(p f) -> p f", p=P),
                        rp[:])
                    rc = brdp.tile([64, QH], F32, tag="rc")
                    nc.sync.dma_start(
                        rc[:], rsums_d[h:h + 1, q0:q0 + QH].broadcast_to((64, QH)))
                    nc.vector.tensor_tensor(
                        attn_sb[ph:ph + 64, mh, q0:q0 + QH],
                        attsb[0:64, :], rc[:], op=ALU.mult)

            pending_tail = None
            for bi, (mh, jh) in enumerate([(m, j) for m in range(2)
                                           for j in range(2)]):
                q0 = jh * QH
                att_A = attp.tile([65, QH], F32, tag="att", name="att_A")
                att_B = attp.tile([65, QH], F32, tag="att", name="att_B")
                for i in range(TC):
                    st_A = stp.tile([P, QH], F32, tag="st", name="st_A")
                    st_B = stp.tile([P, QH], F32, tag="st", name="st_B")
                    for n in range(2):
                        ns = slice(n * 512, (n + 1) * 512)
                        qs = slice(q0 + n * 512, q0 + (n + 1) * 512)
                        nc.tensor.matmul(
                            st_A[:, ns],
                            lhsT=kt_sb[0:64, mh, i * P:(i + 1) * P],
                            rhs=qt_sb[0:64, mh, qs],
                            start=True, stop=True)
                        nc.tensor.matmul(
                            st_B[:, ns],
                            lhsT=kt_sb[64:128, mh, i * P:(i + 1) * P],
                            rhs=qt_sb[64:128, mh, qs],
                            start=True, stop=True)
                    if bi == 0 and i == 0:
                        # Full-row keep-warm matmuls: bridge the exp
                        # pipeline-fill stall so the PE's HAM activity window
                        # never sees >3.4us idle (which would gate the clock
                        # to 1.2 GHz for the whole phase). The i=0 PV has
                        # start=True, which overwrites this garbage.
                        for wmm in range(8):
                            nc.tensor.matmul(
                                att_A[:, (wmm % 2) * 512:(wmm % 2) * 512 + 512],
                                lhsT=v_sb[:, 0, 2 * mh, 0:65],
                                rhs=kt_sb[:, mh, 0:512],
                                start=True, stop=True)
                    pt_A = ptp.tile([P, QH], BF16, tag="pt", name="pt_A")
                    nc.scalar.activation(pt_A[:], st_A[:], AT.Exp,
                                         bias=zero_sb[:, 0:1], scale=float(SCALE))
                    pt_B = ptp.tile([P, QH], BF16, tag="pt", name="pt_B")
                    nc.scalar.activation(pt_B[:], st_B[:], AT.Exp,
                                         bias=zero_sb[:, 0:1], scale=float(SCALE))
                    pending_pv.append((att_A, att_B, mh, i, pt_A, pt_B))
                    if len(pending_pv) > 1:
                        emit_pv(pending_pv.pop(0))
                    if i == 1 and pending_tail is not None:
                        emit_block_tail(pending_tail)
                        pending_tail = None
                pending_tail = (att_A, att_B, mh, jh)
            while pending_pv:
                emit_pv(pending_pv.pop(0))
            emit_block_tail(pending_tail)

        # ---------------- Phase C: output projection (partial) ----------------
        with tc.tile_pool(name="ops", bufs=3, space="PSUM") as ops, \
             tc.tile_pool(name="owm", bufs=1, space="PSUM") as owm, \
             tc.tile_pool(name="osb", bufs=4) as osb:
            wmt = owm.tile([P, 512], F32, tag="wmt")
            for m in range(TC):
                po = ops.tile([P, D], F32, tag="po")
                # keep-warm matmul: the evac-paced pipeline here has PE gaps
                # that otherwise let HAM re-throttle the clock
                nc.tensor.matmul(wmt[:], lhsT=attn_sb[:, 0, 0:P],
                                 rhs=wot_sb[:, 0, 0:512], start=True, stop=True)
                for sc in range(2):
                    for n in range(2):
                        nc.tensor.matmul(
                            po[:, n * 512:(n + 1) * 512],
                            lhsT=attn_sb[:, sc, m * P:(m + 1) * P],
                            rhs=wot_sb[:, sc, n * 512:(n + 1) * 512],
                            start=(sc == 0), stop=(sc == 1))
                ob = osb.tile([P, D], BF16, tag="ob")
                if m % 2 == 0:
                    nc.scalar.copy(ob[:], po[:])
                else:
                    nc.vector.tensor_copy(ob[:], po[:])
                nc.sync.dma_start(out_d[m], ob[:])


def _shard_inputs(query, key, value, wq, bq, wk, bk, wv, bv, wo):
    """Build the 8 per-core input maps (all host-side numpy)."""
    bf16 = ml_dtypes.bfloat16
    in_maps = []

    def fold_dmajor(a_t, inner):
        # (D, inner) -> [P, DC, inner]
        return np.ascontiguousarray(
            a_t.reshape(DC, P, inner).transpose(1, 0, 2))

    xs = {}
    for b in range(B):
        for name, x in (("xq", query), ("xk", key), ("xv", value)):
            xt = np.ascontiguousarray(x[b].T).astype(bf16)  # (D, T)
            xs[(name, b)] = fold_dmajor(xt, T)

    for c in range(N_CORES):
        b, g = divmod(c, NHL)
        gs = g * S
        wq_g = wq[gs:gs + S]          # (S, D)
        wk_g = wk[gs:gs + S]
        wv_g = wv[gs:gs + S]
        wo_g = wo[:, gs:gs + S]       # (D, S)
        m = {
            "xq": xs[("xq", b)],
            "xk": xs[("xk", b)],
            "xv": xs[("xv", b)],
            "wqt": fold_dmajor(np.ascontiguousarray(wq_g.T).astype(bf16), S),
            "wkt": fold_dmajor(np.ascontiguousarray(wk_g.T).astype(bf16), S),
            "wvt": fold_dmajor(np.ascontiguousarray(wv_g.T).astype(bf16), S),
            "bq": np.ascontiguousarray(
                bq[gs:gs + S].reshape(2, P).T).astype(np.float32),
            "bk": np.ascontiguousarray(
                bk[gs:gs + S].reshape(2, P).T).astype(np.float32),
            "bv": np.ascontiguousarray(np.broadcast_to(
                bv[gs:gs + S].reshape(NHL, HEAD_DIM), (P, NHL, HEAD_DIM))
            ).astype(np.float32),
            "wot": np.ascontiguousarray(
                wo_g.T.reshape(2, P, D).transpose(1, 0, 2)).astype(bf16),
        }
        in_maps.append(m)
    return in_maps


def _reference_numpy(query, key, value, mask, wq, bq, wk, bk, wv, bv, wo, bo):
    """Pure-numpy fallback for non-trivial masks (never hit for spec inputs)."""
    def lin(x, w, b):
        return np.einsum("btd,od->bto", x, w) + b
    Bq, Tq, _ = query.shape
    Q = lin(query, wq, bq).reshape(Bq, Tq, N_HEADS, HEAD_DIM).transpose(0, 2, 1, 3)
    K = lin(key, wk, bk).reshape(Bq, Tq, N_HEADS, HEAD_DIM).transpose(0, 2, 1, 3)
    V = lin(value, wv, bv).reshape(Bq, Tq, N_HEADS, HEAD_DIM).transpose(0, 2, 1, 3)
    scores = np.einsum("bhqd,bhkd->bhqk", Q, K) * SCALE
    scores = np.where(mask[:, None, :, :] == 0, -np.inf, scores)
    scores = scores - scores.max(axis=-1, keepdims=True)
    e = np.exp(scores)
    probs = e / e.sum(axis=-1, keepdims=True)
    att = np.einsum("bhqk,bhkd->bhqd", probs, V)
    att = att.transpose(0, 2, 1, 3).reshape(Bq, Tq, N_HEADS * HEAD_DIM)
    return (np.einsum("btd,od->bto", att, wo) + bo).astype(np.float32)


def _enable_local_tracing():
    """Make bass_utils' axon NTFF-trace path work in this container:
    register the ctypes profile hook under the missing antenv.axon_hooks
    name and keep artifacts local instead of uploading."""
    import sys
    import types
    try:
        import antenv.axon_hooks  # noqa: F401
    except Exception:
        try:
            from trn_agent_boot.trn_boot import _ntff_profile_via_ctypes
            hook = _ntff_profile_via_ctypes("/opt/axon/libaxon_pjrt.so")
            if hook is None:
                return False
            holder = {"hook": hook}
            m2 = types.ModuleType("antenv.axon_hooks")
            m2.get_axon_ntff_profile_hook = lambda: holder["hook"]
            m2.set_axon_ntff_profile_hook = lambda h: holder.update(hook=h)
            if "antenv" not in sys.modules:
                m1 = types.ModuleType("antenv")
                m1.axon_hooks = m2
                sys.modules["antenv"] = m1
            else:
                sys.modules["antenv"].axon_hooks = m2
            sys.modules["antenv.axon_hooks"] = m2
        except Exception:
            return False
    bass_utils.upload_artifacts = lambda tmpdir: tmpdir
    return True


def kernel(query, key, value, mask, wq, bq, wk, bk, wv, bv, wo, bo):
    query = np.asarray(query, np.float32)
    key = np.asarray(key, np.float32)
    value = np.asarray(value, np.float32)
    wq_, bq_ = np.asarray(wq, np.float32), np.asarray(bq, np.float32)
    wk_, bk_ = np.asarray(wk, np.float32), np.asarray(bk, np.float32)
    wv_, bv_ = np.asarray(wv, np.float32), np.asarray(bv, np.float32)
    wo_, bo_ = np.asarray(wo, np.float32), np.asarray(bo, np.float32)
    mask_np = np.asarray(mask)

    if not np.all(mask_np != 0):
        # Spec inputs always have an all-ones mask; keep a correct fallback.
        return _reference_numpy(query, key, value, mask_np, wq_, bq_,
                                wk_, bk_, wv_, bv_, wo_, bo_)

    # Experimental only: walrus's LDW opt rejects some of our weight loads.
    if os.environ.get("KERNEL_LDW_OPT", "0") == "1":
        _patch_walrus_flags()

    if "prog" not in _CACHE:
        _CACHE["prog"] = _build_program()
    nc = _CACHE["prog"]

    in_maps = _shard_inputs(query, key, value, wq_, bq_, wk_, bk_, wv_, bv_, wo_)

    trace = os.environ.get("KERNEL_TRACE", "0") == "1"
    kw = {}
    if trace:
        trace = _enable_local_tracing()
        if trace:
            tdir = os.environ.get("KERNEL_TRACE_DIR")
            if tdir:
                os.makedirs(tdir, exist_ok=True)
                kw["tmpdir"] = tdir
    try:
        res = bass_utils.run_bass_kernel_spmd(
            nc, in_maps, core_ids=list(range(N_CORES)), trace=trace, **kw)
    except Exception:
        if not trace:
            raise
        import traceback
        traceback.print_exc()
        res = bass_utils.run_bass_kernel_spmd(
            nc, in_maps, core_ids=list(range(N_CORES)), trace=False)

    LAST_STATS.clear()
    LAST_STATS["exec_time_ns"] = res.exec_time_ns
    LAST_STATS["profile_json"] = res.profile_json
    if res.instructions_and_trace is not None:
        LAST_STATS["trace_url"] = res.instructions_and_trace[1]

    out = np.empty((B, T, D), np.float32)
    for b in range(B):
        acc = np.zeros((T, D), np.float32)
        for g in range(NHL):
            acc += res.results[b * NHL + g]["out_part"].reshape(T, D).astype(
                np.float32)
        out[b] = acc + bo_
    return out


# revision 41
# speedup vs baseline: 1.0200x; 1.0179x over previous
"""Multi-head attention (COAMultiHeadAttention) on 8 Trainium2 NeuronCores.

Sharding: batch x head-group. Core c (0..7) handles batch b = c//4 and head
group g = c%4 (4 of 16 heads, i.e. a 256-wide slice of the 1024-dim model).
Each core:
  - projects its batch's q/k/v through its 256-row weight slices (Q^T/K^T
    d-major, V token-major),
  - runs attention for its 4 heads in a score-transposed (k-part, q-free)
    layout: QK^T -> exp on ScalarE (PSUM->SBUF, bf16) -> P~V with an extra
    ones column in V that yields the softmax denominators for free,
  - normalizes via GPSIMD partition-broadcast + VectorE reciprocal/multiply,
  - computes its partial output projection att_n @ wo[:, slice]^T.
Host sums the 4 partials per batch and adds bo.

Everything the device sees is pre-laid-out by the host (transposes, head
slicing, bias replication), so the device program is identical on all cores.
"""

import os

import ml_dtypes
import numpy as np

import concourse.bass as bass
import concourse.mybir as mybir
import concourse.tile as tile
from concourse import bacc, bass_utils

F32 = mybir.dt.float32
F32R = mybir.dt.float32r
BF16 = mybir.dt.bfloat16
AT = mybir.ActivationFunctionType
ALU = mybir.AluOpType

B = 2
T = 2048
D = 1024
N_HEADS = 16
HEAD_DIM = 64
N_CORES = 8
S = 256            # per-core slice of the model dim (4 heads)
NHL = 4            # heads per core
P = 128
DC = D // P        # 8 contraction chunks for the projections
TC = T // P        # 16 token chunks
SCALE = 1.0 / np.sqrt(HEAD_DIM)

_CACHE = {}
LAST_STATS = {}


def _patch_walrus_flags():
    """Enable walrus's LDWEIGHTS optimization (dedupe/pipeline weight loads).
    bass_utils hardcodes --enable-ldw-opt=false; without the opt every MATMUL
    serially waits ~140ns for its own LDWEIGHTS, which costs ~100us here."""
    if getattr(bass_utils, "_ldw_patched", False):
        return
    orig = bass_utils.run_command

    def patched(argv, **kw):
        argv = ["--enable-ldw-opt=true" if a == "--enable-ldw-opt=false" else a
                for a in argv]
        return orig(argv, **kw)

    bass_utils.run_command = patched
    bass_utils._ldw_patched = True


def _build_program():
    nc = bacc.Bacc("TRN2", target_bir_lowering=False, debug=False)

    xq_d = nc.dram_tensor("xq", [P, DC, T], BF16, kind="ExternalInput").ap()
    xk_d = nc.dram_tensor("xk", [P, DC, T], BF16, kind="ExternalInput").ap()
    xv_d = nc.dram_tensor("xv", [P, DC, T], BF16, kind="ExternalInput").ap()
    wqt_d = nc.dram_tensor("wqt", [P, DC, S], BF16, kind="ExternalInput").ap()
    wkt_d = nc.dram_tensor("wkt", [P, DC, S], BF16, kind="ExternalInput").ap()
    wvt_d = nc.dram_tensor("wvt", [P, DC, S], BF16, kind="ExternalInput").ap()
    bq_d = nc.dram_tensor("bq", [P, 2], F32, kind="ExternalInput").ap()
    bk_d = nc.dram_tensor("bk", [P, 2], F32, kind="ExternalInput").ap()
    bv_d = nc.dram_tensor("bv", [P, NHL, HEAD_DIM], F32, kind="ExternalInput").ap()
    wot_d = nc.dram_tensor("wot", [P, 2, D], BF16, kind="ExternalInput").ap()
    out_d = nc.dram_tensor("out_part", [TC, P, D], BF16, kind="ExternalOutput").ap()
    sums_d = nc.dram_tensor("sums_scr", [NHL, T], F32).ap()
    rsums_d = nc.dram_tensor("rsums_scr", [NHL, T], F32).ap()

    with tile.TileContext(nc) as tc:
        _body(tc, xq_d, xk_d, xv_d, wqt_d, wkt_d, wvt_d,
              bq_d, bk_d, bv_d, wot_d, out_d, sums_d, rsums_d)
    nc.compile()
    return nc


def _body(tc, xq_d, xk_d, xv_d, wqt_d, wkt_d, wvt_d, bq_d, bk_d, bv_d, wot_d,
          out_d, sums_d, rsums_d):
    nc = tc.nc

    from contextlib import ExitStack
    with ExitStack() as ctx:
        pers = ctx.enter_context(tc.tile_pool(name="pers", bufs=1))
        qt_sb = pers.tile([P, 2, T], BF16, tag="qt")
        kt_sb = pers.tile([P, 2, T], BF16, tag="kt")
        v_sb = pers.tile([P, TC, NHL, 68], BF16, tag="v")
        attn_sb = pers.tile([P, 2, T], BF16, tag="attn")
        wot_sb = pers.tile([P, 2, D], BF16, tag="wot")
        bq_sb = pers.tile([P, 2], F32, tag="bq")
        bk_sb = pers.tile([P, 2], F32, tag="bk")
        bv_sb = pers.tile([P, NHL, HEAD_DIM], F32, tag="bv")
        zero_sb = pers.tile([P, 1], F32, tag="zero")
        scr_sb = pers.tile([P, 1], F32, tag="scr")

        nc.sync.dma_start(wot_sb[:], wot_d[:])
        nc.sync.dma_start(bq_sb[:], bq_d[:])
        nc.sync.dma_start(bk_sb[:], bk_d[:])
        nc.sync.dma_start(bv_sb[:], bv_d[:])
        nc.vector.memset(zero_sb[:], 0.0)
        # Preload the exp table set (~2.7us) during phase A so the first real
        # exp doesn't stall the attention pipeline right when HAM's idle
        # window could re-throttle the PE clock.
        nc.scalar.activation(scr_sb[:], zero_sb[:], AT.Exp,
                             bias=zero_sb[:, 0:1], scale=1.0)
        # ones column for the P~V sum trick (col 64 of every (tchunk, head) slot)
        nc.vector.memset(v_sb[:, :, :, 64:65], 1.0)

        # ---------------- Phase A: projections ----------------
        with tc.tile_pool(name="xw", bufs=1) as xw, \
             tc.tile_pool(name="pjps", bufs=4, space="PSUM") as pjps:
            wq_sb = xw.tile([P, DC, S], BF16, tag="wq")
            wk_sb = xw.tile([P, DC, S], BF16, tag="wk")
            wv_sb = xw.tile([P, DC, S], BF16, tag="wv")
            xq_sb = xw.tile([P, DC, T], BF16, tag="xq")
            xk_sb = xw.tile([P, DC, T], BF16, tag="xk")
            xv_sb = xw.tile([P, DC, T], BF16, tag="xv")

            for c in range(DC):
                nc.sync.dma_start(wk_sb[:, c], wkt_d[:, c])
                nc.sync.dma_start(xk_sb[:, c], xk_d[:, c])
            for c in range(DC):
                nc.sync.dma_start(wq_sb[:, c], wqt_d[:, c])
                nc.sync.dma_start(xq_sb[:, c], xq_d[:, c])
            for c in range(DC):
                nc.sync.dma_start(wv_sb[:, c], wvt_d[:, c])
                nc.sync.dma_start(xv_sb[:, c], xv_d[:, c])

            # K^T and Q^T projections: (256, T) d-major, bf16 + bias.
            # d8-outer so compute starts as soon as the first 128-row chunk
            # of x lands, with all four n-groups accumulating in PSUM.
            def proj_kq(x_sb, w_sb, b_sb, dst, m):
                for n in range(4):
                    ps = pjps.tile([P, 512], F32, tag="pj")
                    for d8 in range(DC):
                        nc.tensor.matmul(
                            ps[:],
                            lhsT=w_sb[:, d8, m * P:(m + 1) * P],
                            rhs=x_sb[:, d8, n * 512:(n + 1) * 512],
                            start=(d8 == 0), stop=(d8 == DC - 1))
                    nc.vector.tensor_scalar(
                        dst[:, m, n * 512:(n + 1) * 512], ps[:],
                        b_sb[:, m:m + 1], None, op0=ALU.add)

            proj_kq(xk_sb, wk_sb, bk_sb, kt_sb, 0)
            proj_kq(xk_sb, wk_sb, bk_sb, kt_sb, 1)
            proj_kq(xq_sb, wq_sb, bq_sb, qt_sb, 0)
            proj_kq(xq_sb, wq_sb, bq_sb, qt_sb, 1)

            # V projection: token-major (T, 256) bf16 + bias
            for t16 in range(TC):
                ps = pjps.tile([P, S], F32, tag="pj")
                for d8 in range(DC):
                    nc.tensor.matmul(
                        ps[:],
                        lhsT=xv_sb[:, d8, t16 * P:(t16 + 1) * P],
                        rhs=wv_sb[:, d8, :],
                        start=(d8 == 0), stop=(d8 == DC - 1))
                nc.vector.tensor_tensor(
                    v_sb[:, t16, :, 0:64],
                    ps[:].rearrange("p (h x) -> p h x", h=NHL),
                    bv_sb[:], op=ALU.add)

        # ---------------- Phase B: attention ----------------
        # Heads are processed in pairs (0,1) and (2,3). Within a pair, head A
        # lives at partitions 0-63 and head B at 64-127, so interleaved QK
        # matmuls alternate PE row groups — the PE's reorder window can then
        # pull each LDWEIGHTS ahead of the other head's in-flight matmul.
        # q is processed in halves of 1024 so the four PSUM accumulators
        # (2 heads x (ST tile + att tile)) fit in the 8 banks exactly.
        # The PV matmuls lag one step behind QK/exp, and each block's final
        # PV + evacuations are emitted after the NEXT block's first QK/exp —
        # so the ScalarE exp stream never stalls at block boundaries.
        QH = 1024
        with tc.tile_pool(name="stp", bufs=2, space="PSUM") as stp, \
             tc.tile_pool(name="attp", bufs=2, space="PSUM") as attp, \
             tc.tile_pool(name="ptp", bufs=6) as ptp, \
             tc.tile_pool(name="asb", bufs=4) as asbp, \
             tc.tile_pool(name="brd", bufs=2) as brdp, \
             tc.tile_pool(name="rcp", bufs=4) as rcpp:
            pending_pv = []

            def emit_pv(ent):
                att_A, att_B, mh, i, pt_A, pt_B = ent
                for n in range(2):
                    ns = slice(n * 512, (n + 1) * 512)
                    nc.tensor.matmul(
                        att_A[:, ns], lhsT=v_sb[:, i, 2 * mh, 0:65],
                        rhs=pt_A[:, ns], start=(i == 0), stop=(i == TC - 1))
                    nc.tensor.matmul(
                        att_B[:, ns], lhsT=v_sb[:, i, 2 * mh + 1, 0:65],
                        rhs=pt_B[:, ns], start=(i == 0), stop=(i == TC - 1))

            def emit_block_tail(ent):
                att_A, att_B, mh, jh = ent
                q0 = jh * QH
                # Evacuate both PSUM accumulators first (frees the banks for
                # the next block before the slower norm chains run — DVE is
                # strict FIFO, so norm DMA-waits must not sit between them).
                attsbs = []
                for hb, att_ps in ((0, att_A), (1, att_B)):
                    attsb = asbp.tile([65, QH], F32, tag="attsb",
                                      name=f"attsb{hb}")
                    nc.vector.tensor_copy(attsb[:], att_ps[:])
                    attsbs.append(attsb)
                for hb, attsb in ((0, attsbs[0]), (1, attsbs[1])):
                    h = 2 * mh + hb
                    ph = hb * 64
                    # Softmax denominators: reciprocal in partition-major
                    # shape (DVE cost scales with free size only), then a
                    # partition broadcast — both via DRAM bounces, since
                    # SBUF APs cannot have a zero partition step.
                    nc.sync.dma_start(sums_d[h:h + 1, q0:q0 + QH],
                                      attsb[64:65, :])
                    sp = rcpp.tile([P, QH // P], F32, tag="sp")
                    nc.sync.dma_start(
                        sp[:], sums_d[h, q0:q0 + QH].rearrange(
                            "(p f) -> p f", p=P))
                    rp = rcpp.tile([P, QH // P], F32, tag="rp")
                    nc.vector.reciprocal(rp[:], sp[:])
                    nc.sync.dma_start(
                        rsums_d[h, q0:q0 + QH].rearrange("(p f) -> p f", p=P),
                        rp[:])
                    rc = brdp.tile([64, QH], F32, tag="rc")
                    nc.sync.dma_start(
                        rc[:], rsums_d[h:h + 1, q0:q0 + QH].broadcast_to((64, QH)))
                    nc.vector.tensor_tensor(
                        attn_sb[ph:ph + 64, mh, q0:q0 + QH],
                        attsb[0:64, :], rc[:], op=ALU.mult)

            pending_tail = None
            for bi, (mh, jh) in enumerate([(m, j) for m in range(2)
                                           for j in range(2)]):
                q0 = jh * QH
                att_A = attp.tile([65, QH], F32, tag="att", name="att_A")
                att_B = attp.tile([65, QH], F32, tag="att", name="att_B")
                for i in range(TC):
                    st_A = stp.tile([P, QH], F32, tag="st", name="st_A")
                    st_B = stp.tile([P, QH], F32, tag="st", name="st_B")
                    for n in range(2):
                        ns = slice(n * 512, (n + 1) * 512)
                        qs = slice(q0 + n * 512, q0 + (n + 1) * 512)
                        nc.tensor.matmul(
                            st_A[:, ns],
                            lhsT=kt_sb[0:64, mh, i * P:(i + 1) * P],
                            rhs=qt_sb[0:64, mh, qs],
                            start=True, stop=True)
                        nc.tensor.matmul(
                            st_B[:, ns],
                            lhsT=kt_sb[64:128, mh, i * P:(i + 1) * P],
                            rhs=qt_sb[64:128, mh, qs],
                            start=True, stop=True)
                    if bi == 0 and i == 0:
                        # Full-row keep-warm matmuls: bridge the exp
                        # pipeline-fill stall so the PE's HAM activity window
                        # never sees >3.4us idle (which would gate the clock
                        # to 1.2 GHz for the whole phase). The i=0 PV has
                        # start=True, which overwrites this garbage.
                        for wmm in range(8):
                            nc.tensor.matmul(
                                att_A[:, (wmm % 2) * 512:(wmm % 2) * 512 + 512],
                                lhsT=v_sb[:, 0, 2 * mh, 0:65],
                                rhs=kt_sb[:, mh, 0:512],
                                start=True, stop=True)
                    pt_A = ptp.tile([P, QH], BF16, tag="pt", name="pt_A")
                    nc.scalar.activation(pt_A[:], st_A[:], AT.Exp,
                                         bias=zero_sb[:, 0:1], scale=float(SCALE))
                    pt_B = ptp.tile([P, QH], BF16, tag="pt", name="pt_B")
                    nc.scalar.activation(pt_B[:], st_B[:], AT.Exp,
                                         bias=zero_sb[:, 0:1], scale=float(SCALE))
                    pending_pv.append((att_A, att_B, mh, i, pt_A, pt_B))
                    if len(pending_pv) > 1:
                        emit_pv(pending_pv.pop(0))
                    if i == 1 and pending_tail is not None:
                        emit_block_tail(pending_tail)
                        pending_tail = None
                pending_tail = (att_A, att_B, mh, jh)
            while pending_pv:
                emit_pv(pending_pv.pop(0))
            emit_block_tail(pending_tail)

        # ---------------- Phase C: output projection (partial) ----------------
        with tc.tile_pool(name="ops", bufs=3, space="PSUM") as ops, \
             tc.tile_pool(name="owm", bufs=1, space="PSUM") as owm, \
             tc.tile_pool(name="osb", bufs=4) as osb:
            wmt = owm.tile([P, 512], F32, tag="wmt")
            for m in range(TC):
                po = ops.tile([P, D], F32, tag="po")
                # keep-warm matmul: the evac-paced pipeline here has PE gaps
                # that otherwise let HAM re-throttle the clock
                nc.tensor.matmul(wmt[:], lhsT=attn_sb[:, 0, 0:P],
                                 rhs=wot_sb[:, 0, 0:512], start=True, stop=True)
                for sc in range(2):
                    for n in range(2):
                        nc.tensor.matmul(
                            po[:, n * 512:(n + 1) * 512],
                            lhsT=attn_sb[:, sc, m * P:(m + 1) * P],
                            rhs=wot_sb[:, sc, n * 512:(n + 1) * 512],
                            start=(sc == 0), stop=(sc == 1))
                ob = osb.tile([P, D], BF16, tag="ob")
                if m % 2 == 0:
                    nc.scalar.copy(ob[:], po[:])
                else:
                    nc.vector.tensor_copy(ob[:], po[:])
                nc.sync.dma_start(out_d[m], ob[:])


def _shard_inputs(query, key, value, wq, bq, wk, bk, wv, bv, wo):
    """Build the 8 per-core input maps (all host-side numpy)."""
    bf16 = ml_dtypes.bfloat16
    in_maps = []

    def fold_dmajor(a_t, inner):
        # (D, inner) -> [P, DC, inner]
        return np.ascontiguousarray(
            a_t.reshape(DC, P, inner).transpose(1, 0, 2))

    xs = {}
    for b in range(B):
        for name, x in (("xq", query), ("xk", key), ("xv", value)):
            xt = np.ascontiguousarray(x[b].T).astype(bf16)  # (D, T)
            xs[(name, b)] = fold_dmajor(xt, T)

    for c in range(N_CORES):
        b, g = divmod(c, NHL)
        gs = g * S
        wq_g = wq[gs:gs + S]          # (S, D)
        wk_g = wk[gs:gs + S]
        wv_g = wv[gs:gs + S]
        wo_g = wo[:, gs:gs + S]       # (D, S)
        m = {
            "xq": xs[("xq", b)],
            "xk": xs[("xk", b)],
            "xv": xs[("xv", b)],
            "wqt": fold_dmajor(np.ascontiguousarray(wq_g.T).astype(bf16), S),
            "wkt": fold_dmajor(np.ascontiguousarray(wk_g.T).astype(bf16), S),
            "wvt": fold_dmajor(np.ascontiguousarray(wv_g.T).astype(bf16), S),
            "bq": np.ascontiguousarray(
                bq[gs:gs + S].reshape(2, P).T).astype(np.float32),
            "bk": np.ascontiguousarray(
                bk[gs:gs + S].reshape(2, P).T).astype(np.float32),
            "bv": np.ascontiguousarray(np.broadcast_to(
                bv[gs:gs + S].reshape(NHL, HEAD_DIM), (P, NHL, HEAD_DIM))
            ).astype(np.float32),
            "wot": np.ascontiguousarray(
                wo_g.T.reshape(2, P, D).transpose(1, 0, 2)).astype(bf16),
        }
        in_maps.append(m)
    return in_maps


def _reference_numpy(query, key, value, mask, wq, bq, wk, bk, wv, bv, wo, bo):
    """Pure-numpy fallback for non-trivial masks (never hit for spec inputs)."""
    def lin(x, w, b):
        return np.einsum("btd,od->bto", x, w) + b
    Bq, Tq, _ = query.shape
    Q = lin(query, wq, bq).reshape(Bq, Tq, N_HEADS, HEAD_DIM).transpose(0, 2, 1, 3)
    K = lin(key, wk, bk).reshape(Bq, Tq, N_HEADS, HEAD_DIM).transpose(0, 2, 1, 3)
    V = lin(value, wv, bv).reshape(Bq, Tq, N_HEADS, HEAD_DIM).transpose(0, 2, 1, 3)
    scores = np.einsum("bhqd,bhkd->bhqk", Q, K) * SCALE
    scores = np.where(mask[:, None, :, :] == 0, -np.inf, scores)
    scores = scores - scores.max(axis=-1, keepdims=True)
    e = np.exp(scores)
    probs = e / e.sum(axis=-1, keepdims=True)
    att = np.einsum("bhqk,bhkd->bhqd", probs, V)
    att = att.transpose(0, 2, 1, 3).reshape(Bq, Tq, N_HEADS * HEAD_DIM)
    return (np.einsum("btd,od->bto", att, wo) + bo).astype(np.float32)


def _enable_local_tracing():
    """Make bass_utils' axon NTFF-trace path work in this container:
    register the ctypes profile hook under the missing antenv.axon_hooks
    name and keep artifacts local instead of uploading."""
    import sys
    import types
    try:
        import antenv.axon_hooks  # noqa: F401
    except Exception:
        try:
            from trn_agent_boot.trn_boot import _ntff_profile_via_ctypes
            hook = _ntff_profile_via_ctypes("/opt/axon/libaxon_pjrt.so")
            if hook is None:
                return False
            holder = {"hook": hook}
            m2 = types.ModuleType("antenv.axon_hooks")
            m2.get_axon_ntff_profile_hook = lambda: holder["hook"]
            m2.set_axon_ntff_profile_hook = lambda h: holder.update(hook=h)
            if "antenv" not in sys.modules:
                m1 = types.ModuleType("antenv")
                m1.axon_hooks = m2
                sys.modules["antenv"] = m1
            else:
                sys.modules["antenv"].axon_hooks = m2
            sys.modules["antenv.axon_hooks"] = m2
        except Exception:
            return False
    bass_utils.upload_artifacts = lambda tmpdir: tmpdir
    return True


def kernel(query, key, value, mask, wq, bq, wk, bk, wv, bv, wo, bo):
    query = np.asarray(query, np.float32)
    key = np.asarray(key, np.float32)
    value = np.asarray(value, np.float32)
    wq_, bq_ = np.asarray(wq, np.float32), np.asarray(bq, np.float32)
    wk_, bk_ = np.asarray(wk, np.float32), np.asarray(bk, np.float32)
    wv_, bv_ = np.asarray(wv, np.float32), np.asarray(bv, np.float32)
    wo_, bo_ = np.asarray(wo, np.float32), np.asarray(bo, np.float32)
    mask_np = np.asarray(mask)

    if not np.all(mask_np != 0):
        # Spec inputs always have an all-ones mask; keep a correct fallback.
        return _reference_numpy(query, key, value, mask_np, wq_, bq_,
                                wk_, bk_, wv_, bv_, wo_, bo_)

    # Experimental only: walrus's LDW opt rejects some of our weight loads.
    if os.environ.get("KERNEL_LDW_OPT", "0") == "1":
        _patch_walrus_flags()

    if "prog" not in _CACHE:
        _CACHE["prog"] = _build_program()
    nc = _CACHE["prog"]

    in_maps = _shard_inputs(query, key, value, wq_, bq_, wk_, bk_, wv_, bv_, wo_)

    trace = os.environ.get("KERNEL_TRACE", "0") == "1"
    kw = {}
    if trace:
        trace = _enable_local_tracing()
        if trace:
            tdir = os.environ.get("KERNEL_TRACE_DIR")
            if tdir:
                os.makedirs(tdir, exist_ok=True)
                kw["tmpdir"] = tdir
    try:
        res = bass_utils.run_bass_kernel_spmd(
            nc, in_maps, core_ids=list(range(N_CORES)), trace=trace, **kw)
    except Exception:
        if not trace:
            raise
        import traceback
        traceback.print_exc()
        res = bass_utils.run_bass_kernel_spmd(
            nc, in_maps, core_ids=list(range(N_CORES)), trace=False)

    LAST_STATS.clear()
    LAST_STATS["exec_time_ns"] = res.exec_time_ns
    LAST_STATS["profile_json"] = res.profile_json
    if res.instructions_and_trace is not None:
        LAST_STATS["trace_url"] = res.instructions_and_trace[1]

    out = np.empty((B, T, D), np.float32)
    for b in range(B):
        acc = np.zeros((T, D), np.float32)
        for g in range(NHL):
            acc += res.results[b * NHL + g]["out_part"].reshape(T, D).astype(
                np.float32)
        out[b] = acc + bo_
    return out
